# revision 1
# baseline (speedup 1.0000x reference)
"""Trainium2 Bass kernel for a 2-layer GAT (nn_GAT_Net): 50k nodes, 800k edges.

Strategy (8 NeuronCores, SPMD):
  - dst-partitioned edge sharding: core c owns dst nodes [c*6250, (c+1)*6250).
  - Phase A (per core): h1 = x_slab @ W1 and attention projections for its
    node slab; writes a gather table tab1 [Np, 320] (interleaved
    [8 x (head(32) | 1.0)] | as1(8) | pad) and a dst-side table adtab1
    [Np, 64] (ad1 replicated 8x).  AllGather tab1 across cores.
  - Phase B (edge phase, layer 1): edges grouped by 128-dst windows; per
    3-window gather group, batched dma_gather of src rows (one call per
    table bank, int16 indices, single_packet=False) + one dma_gather of ad
    rows from the core-LOCAL adtab (dst indices are core-local).  Per 128-edge tile: one-hot(dst) matrix via
    iota/is_equal, msg = e_exp * hsrc (the 1.0 columns turn into e_exp and
    produce the softmax denominator), one accumulating matmul
    OneHot^T @ msg into the window PSUM.  Window tail: normalize by the
    denominator (+1e-16 like the reference), +b1, ELU, then layer-2 node
    projections (h2, as2, ad2) -> tab2 [Np, 64].  AllGather tab2.
  - Phase D (edge phase, layer 2, H=1): same structure; e_exp is folded
    directly into the one-hot (tensor_scalar is_equal*mult), rhs is the raw
    gathered [h2 | 1.0] rows.  Tail: normalize, +b2, log_softmax.
  - Softmax without max-subtraction is exact here (scores bounded ~|7|).

The module is self-contained: only needs numpy + the concourse/bass stack at
/opt/trn_rl_repo (environment infrastructure).
"""
import sys
import os

for _p in ("/opt/trn_rl_repo",):
    if _p not in sys.path:
        sys.path.insert(0, _p)

import numpy as np

from concourse import bass, mybir, bacc
import concourse.tile as tile
from concourse.masks import make_identity
from concourse.bass_utils import run_bass_kernel_spmd

P = 128
FP = mybir.dt.float32
I16 = mybir.dt.int16
I32 = mybir.dt.int32
BF = mybir.dt.bfloat16
AF = mybir.ActivationFunctionType
OP = mybir.AluOpType


class GATConfig:
    def __init__(self, n_nodes=50000, n_edges=800000, n_cores=8, grp=3):
        self.N = n_nodes
        self.E = n_edges
        self.NC = n_cores
        self.F = 256
        self.H = 8
        self.C1 = 32
        self.C2 = 32
        self.SLAB = self.N // self.NC            # real nodes per core
        self.WPC = (self.SLAB + P - 1) // P      # windows per core
        self.SLABP = self.WPC * P                # padded slab
        self.NB = 2                              # src-table banks
        self.BANKP = (self.NC // self.NB) * self.SLABP  # padded rows per bank
        self.ROW1 = 384                          # tab1 row (bf16 elems): 264 h/ones + 16 (as1 f32-as-bf16) + pad
        self.ROW2 = 64                           # tab2 / ad tables row
        self.GRP = grp                           # windows per gather group
        self.groups = [list(range(g, min(g + grp, self.WPC)))
                       for g in range(0, self.WPC, grp)]


def _wrap16(idx):
    """int idx array (len % 128 == 0) -> [128, len//16] int16 wrapped in 16
    partitions, replicated 8x for the Q7 cores."""
    L = len(idx)
    w = np.asarray(idx, np.int16).reshape(L // 16, 16).T  # [16, L//16]
    return np.tile(w, (8, 1))


def preprocess(cfg, edge_index):
    """Partition/sort edges per core; equalize tile counts across cores.

    Returns (geom, per_core) where geom is the shared program structure and
    per_core[c] has the int16/f32 index tensors."""
    src = np.asarray(edge_index[0], np.int64)
    dst = np.asarray(edge_index[1], np.int64)
    NC, SLAB, SLABP, GRP = cfg.NC, cfg.SLAB, cfg.SLABP, cfg.GRP
    BANKP = cfg.BANKP
    srcrow = (src // SLAB) * SLABP + (src % SLAB)  # padded-global row
    bank = (srcrow >= BANKP).astype(np.int64)

    # bucket edges: per core -> per window -> per bank
    edges = [[[None, None] for _ in range(cfg.WPC)] for _ in range(NC)]
    core = dst // SLAB
    for c in range(NC):
        sel = np.nonzero(core == c)[0]
        d_loc = dst[sel] - c * SLAB
        w = d_loc // P
        for wi in range(cfg.WPC):
            wsel = sel[w == wi]
            b = bank[wsel]
            edges[c][wi][0] = wsel[b == 0]
            edges[c][wi][1] = wsel[b == 1]

    # equalized tiles per (group, seg)
    geom = []           # per group: list of (window, bank, nt)
    for g in cfg.groups:
        segs = []
        for wi in g:
            for b in range(cfg.NB):
                cnt = max(len(edges[c][wi][b]) for c in range(NC))
                nt = (cnt + P - 1) // P
                if nt > 0:
                    segs.append((wi, b, nt))
        geom.append(segs)

    per_core = []
    for c in range(NC):
        src_cols, dst_cols, dl_cols = [], [], []
        for gi, g in enumerate(cfg.groups):
            d_rows_g, dl_g = [], []
            for (wi, b, nt) in geom[gi]:
                e = edges[c][wi][b]
                npad = nt * P - len(e)
                sr = srcrow[e] - b * BANKP
                sr = np.concatenate([sr, np.zeros(npad, np.int64)])
                src_cols.append(_wrap16(sr))
                dr = dst[e] - c * SLAB          # local dst row
                dr = np.concatenate([dr, np.zeros(npad, np.int64)])
                d_rows_g.append(dr)
                dlv = (dst[e] - c * SLAB) % P
                dlv = np.concatenate(
                    [dlv.astype(np.float64), np.full(npad, 999.0)])
                dl_g.append(dlv.reshape(nt, P).T)   # [128, nt]
            d_rows_g = np.concatenate(d_rows_g)
            dst_cols.append(_wrap16(d_rows_g))
            dl_cols.append(np.concatenate(dl_g, axis=1))
        per_core.append({
            "srcw": np.concatenate(src_cols, axis=1).astype(np.int16),
            "dstw": np.concatenate(dst_cols, axis=1).astype(np.int16),
            "dstloc": np.concatenate(dl_cols, axis=1).astype(np.float32),
        })
    return geom, per_core


def build_program(cfg, geom):
    ABL = set(os.environ.get("GAT_ABLATE", "").split(","))
    NC, H, C1, C2, F = cfg.NC, cfg.H, cfg.C1, cfg.C2, cfg.F
    ROW1, ROW2, SLABP, WPC = cfg.ROW1, cfg.ROW2, cfg.SLABP, cfg.WPC
    NPT = NC * SLABP
    TT_total = sum(nt for segs in geom for (_, _, nt) in segs)

    nc = bacc.Bacc(None, target_bir_lowering=False, num_devices=NC)

    x_slab = nc.dram_tensor("x_slab", [SLABP, F], FP, kind="ExternalInput")
    W1 = nc.dram_tensor("W1", [F, H * C1], FP, kind="ExternalInput")
    WA1 = nc.dram_tensor("WA1", [F, 2 * H], FP, kind="ExternalInput")
    b1rep = nc.dram_tensor("b1rep", [P, H * C1], FP, kind="ExternalInput")
    W2 = nc.dram_tensor("W2", [H * C1, C2], FP, kind="ExternalInput")
    WA2 = nc.dram_tensor("WA2", [H * C1, 2], FP, kind="ExternalInput")
    b2rep = nc.dram_tensor("b2rep", [P, C2], FP, kind="ExternalInput")
    srcw = nc.dram_tensor("srcw", [P, TT_total * 8], I16, kind="ExternalInput")
    dstw = nc.dram_tensor("dstw", [P, TT_total * 8], I16, kind="ExternalInput")
    dstloc = nc.dram_tensor("dstloc", [P, TT_total], FP, kind="ExternalInput")
    out_d = nc.dram_tensor("out", [SLABP, C2], FP, kind="ExternalOutput")

    with tile.TileContext(nc) as tc:
        with (
            tc.tile_pool(name="sbuf", bufs=1) as sb,
            tc.tile_pool(name="psum", bufs=1, space="PSUM") as ps,
            tc.tile_pool(name="dram", bufs=1, space="DRAM") as dp,
        ):
            # ---- persistent DRAM intermediates ----
            tab1_loc = dp.tile([SLABP, ROW1], BF)
            tab1_full = dp.tile([NPT, ROW1], BF, addr_space="Shared")
            adtab1 = dp.tile([SLABP, ROW2], FP)
            tab2_loc = dp.tile([SLABP, ROW2], FP)
            tab2_full = dp.tile([NPT, ROW2], FP, addr_space="Shared")
            adtab2 = dp.tile([SLABP, ROW2], FP)

            # ---- constants ----
            ident = sb.tile([P, P], FP, tag="ident")
            make_identity(nc, ident[:])
            iota_i = sb.tile([P, P], I32, tag="iota_i")
            nc.gpsimd.iota(iota_i[:], pattern=[[1, P]], base=0,
                           channel_multiplier=0)
            iota_f = sb.tile([P, P], FP, tag="iota_f")
            nc.vector.tensor_copy(iota_f[:], iota_i[:])
            iota_b = sb.tile([P, P], BF, tag="iota_b")
            nc.vector.tensor_copy(iota_b[:], iota_i[:])
            W1sb = sb.tile([P, 2, F], FP, tag="W1sb")
            nc.sync.dma_start(out=W1sb[:, 0, :], in_=W1[0:P, :])
            nc.sync.dma_start(out=W1sb[:, 1, :], in_=W1[P:2 * P, :])
            WA1sb = sb.tile([P, 2, 2 * H], FP, tag="WA1sb")
            nc.sync.dma_start(out=WA1sb[:, 0, :], in_=WA1[0:P, :])
            nc.sync.dma_start(out=WA1sb[:, 1, :], in_=WA1[P:2 * P, :])
            W2sb = sb.tile([P, 2, C2], FP, tag="W2sb")
            nc.sync.dma_start(out=W2sb[:, 0, :], in_=W2[0:P, :])
            nc.sync.dma_start(out=W2sb[:, 1, :], in_=W2[P:2 * P, :])
            WA2sb = sb.tile([P, 2, 2], FP, tag="WA2sb")
            nc.sync.dma_start(out=WA2sb[:, 0, :], in_=WA2[0:P, :])
            nc.sync.dma_start(out=WA2sb[:, 1, :], in_=WA2[P:2 * P, :])
            b1sb = sb.tile([P, H * C1], FP, tag="b1sb")
            nc.sync.dma_start(out=b1sb[:], in_=b1rep[:])
            b2sb = sb.tile([P, C2], FP, tag="b2sb")
            nc.sync.dma_start(out=b2sb[:], in_=b2rep[:])

            # ================= Phase A: node phase, layer 1 =================
            for i in range(WPC):
                r0 = i * P
                xx = sb.tile([P, F], FP, tag="xx", bufs=2)
                nc.sync.dma_start(out=xx[:], in_=x_slab[r0:r0 + P, :])
                xT = sb.tile([P, 2, P], FP, tag="xT", bufs=2)
                for ch in range(2):
                    pt = ps.tile([P, P], FP, tag="tpose", space="PSUM", bufs=2)
                    nc.tensor.transpose(out=pt[:], in_=xx[:, ch * P:(ch + 1) * P],
                                        identity=ident[:])
                    nc.scalar.copy(out=xT[:, ch, :], in_=pt[:])
                hps = ps.tile([P, H * C1], FP, tag="bigps", space="PSUM", bufs=3)
                aps = ps.tile([P, 2 * H], FP, tag="sm1", space="PSUM", bufs=2)
                for ch in range(2):
                    nc.tensor.matmul(out=hps[:], lhsT=xT[:, ch, :],
                                     rhs=W1sb[:, ch, :],
                                     start=(ch == 0), stop=(ch == 1))
                    nc.tensor.matmul(out=aps[:], lhsT=xT[:, ch, :],
                                     rhs=WA1sb[:, ch, :],
                                     start=(ch == 0), stop=(ch == 1))
                t1 = sb.tile([P, ROW1], BF, tag="t1", bufs=2)
                nc.gpsimd.memset(t1[:, 264 + 2 * H:ROW1], 0.0)
                t1v = t1[:, 0:264].rearrange("p (h q) -> p h q", h=H)
                nc.vector.tensor_copy(
                    out=t1v[:, :, 0:C1],
                    in_=hps[:].rearrange("p (h c) -> p h c", h=H))
                nc.gpsimd.memset(t1v[:, :, C1:C1 + 1], 1.0)
                nc.vector.tensor_copy(
                    out=t1[:, 264:264 + 2 * H].bitcast(FP), in_=aps[:, 0:H])
                ad1t = sb.tile([P, ROW2], FP, tag="ad1t", bufs=2)
                nc.vector.tensor_copy(
                    out=ad1t[:].rearrange("p (r h) -> p r h", h=H),
                    in_=aps[:, None, H:2 * H].to_broadcast([P, ROW2 // H, H]))
                nc.sync.dma_start(out=tab1_loc[r0:r0 + P, :], in_=t1[:])
                nc.sync.dma_start(out=adtab1[r0:r0 + P, :], in_=ad1t[:])

            nc.gpsimd.collective_compute(
                "AllGather", OP.bypass,
                replica_groups=[list(range(NC))],
                ins=[tab1_loc[:]], outs=[tab1_full[:]],
            )

            # ============ Phase B: edge phase layer 1 + node phase layer 2 ==
            sc0 = 0   # col offset into srcw (units of 8 cols per tile)
            t0 = 0    # tile offset
            for gi, segs in enumerate(geom):
                TG = sum(nt for (_, _, nt) in segs)
                wins = sorted({wi for (wi, _, _) in segs})
                sidx = sb.tile([P, TG * 8], I16, tag="sidx", bufs=2)
                nc.sync.dma_start(out=sidx[:],
                                  in_=srcw[:, sc0 * 8:(sc0 + TG) * 8])
                didx = sb.tile([P, TG * 8], I16, tag="didx", bufs=2)
                nc.sync.dma_start(out=didx[:],
                                  in_=dstw[:, sc0 * 8:(sc0 + TG) * 8])
                dl = sb.tile([P, TG], FP, tag="dl", bufs=2)
                nc.sync.dma_start(out=dl[:], in_=dstloc[:, t0:t0 + TG])

                gbuf = sb.tile([P, TG * ROW1], BF, tag="gbuf", bufs=2)
                gv = gbuf[:].rearrange("p (t e) -> p t e", e=ROW1)
                off = 0
                for (wi, b, nt) in segs:
                    if "gather" in ABL:
                        nc.sync.dma_start(out=gv[:, off:off + nt, :],
                                          in_=tab1_full[0:P * nt, :].rearrange("(t p) e -> p t e", p=P))
                    else:
                        nc.gpsimd.dma_gather(
                            out_ap=gv[:, off:off + nt, :],
                            in_ap=tab1_full[b * cfg.BANKP:(b + 1) * cfg.BANKP, :],
                            idxs_ap=sidx[:, off * 8:(off + nt) * 8],
                            num_idxs=nt * P, num_idxs_reg=nt * P,
                            elem_size=ROW1, single_packet=False)
                    off += nt
                dbuf = sb.tile([P, TG * ROW2], FP, tag="dbuf", bufs=2)
                dv = dbuf[:].rearrange("p (t e) -> p t e", e=ROW2)
                if "gather" in ABL:
                    nc.sync.dma_start(out=dv, in_=adtab1[0:P * TG, :].rearrange("(t p) e -> p t e", p=P))
                else:
                    nc.gpsimd.dma_gather(
                        out_ap=dv, in_ap=adtab1[:], idxs_ap=didx[:],
                        num_idxs=TG * P, num_idxs_reg=TG * P, elem_size=ROW2, single_packet=False)

                # scores for the whole group
                e_t = sb.tile([P, TG * H], FP, tag="e_t", bufs=2)
                nc.vector.tensor_tensor(
                    out=e_t[:].rearrange("p (t h) -> p t h", h=H),
                    in0=gv[:, :, 264:264 + 2 * H].bitcast(FP),
                    in1=dv[:, :, 0:H], op=OP.add)
                ee = sb.tile([P, TG * H], FP, tag="ee", bufs=2)
                nc.vector.scalar_tensor_tensor(
                    out=ee[:], in0=e_t[:], scalar=0.2, in1=e_t[:],
                    op0=OP.mult, op1=OP.max)
                eex = sb.tile([P, TG * H], FP, tag="eex", bufs=2)
                nc.scalar.activation(out=eex[:], in_=ee[:], func=AF.Exp)
                eexv = eex[:].rearrange("p (t h) -> p t h", h=H)

                # per-window accumulation
                seg_starts = np.cumsum([0] + [nt for (_, _, nt) in segs])
                for wi in wins:
                    tiles = []
                    for si, (w2, b, nt) in enumerate(segs):
                        if w2 == wi:
                            tiles.extend(range(seg_starts[si],
                                               seg_starts[si] + nt))
                    aggps = ps.tile([P, 264], FP, tag="bigps", space="PSUM", bufs=3)
                    for j, t in enumerate(tiles):
                        oh = sb.tile([P, P], BF, tag="oh", bufs=4)
                        nc.vector.tensor_scalar(
                            out=oh[:], in0=iota_b[:], scalar1=dl[:, t:t + 1],
                            scalar2=None, op0=OP.is_equal)
                        msg = sb.tile([P, 264], BF, tag="msg", bufs=4)
                        nc.vector.tensor_tensor(
                            out=msg[:].rearrange("p (h q) -> p h q", h=H),
                            in0=gv[:, t, 0:264].rearrange(
                                "p (h q) -> p h q", h=H),
                            in1=eexv[:, t, :, None].to_broadcast(
                                [P, H, C1 + 1]),
                            op=OP.mult)
                        nc.tensor.matmul(
                            out=aggps[:], lhsT=oh[:], rhs=msg[:],
                            start=(j == 0), stop=(j == len(tiles) - 1))
                    # ---- window tail: normalize + b1 + ELU ----
                    aggv = aggps[:].rearrange("p (h q) -> p h q", h=H)
                    den = sb.tile([P, H], FP, tag="den", bufs=2)
                    nc.vector.tensor_scalar(
                        out=den[:], in0=aggv[:, :, C1], scalar1=1e-16,
                        scalar2=None, op0=OP.add)
                    rden = sb.tile([P, H], FP, tag="rden", bufs=2)
                    nc.vector.reciprocal(out=rden[:], in_=den[:])
                    xb = sb.tile([P, H * C1], FP, tag="xb", bufs=2)
                    nc.vector.tensor_tensor(
                        out=xb[:].rearrange("p (h c) -> p h c", h=H),
                        in0=aggv[:, :, 0:C1],
                        in1=rden[:, :, None].to_broadcast([P, H, C1]),
                        op=OP.mult)
                    nc.vector.tensor_tensor(out=xb[:], in0=xb[:], in1=b1sb[:],
                                            op=OP.add)
                    mn = sb.tile([P, H * C1], FP, tag="mn", bufs=2)
                    nc.vector.tensor_scalar(
                        out=mn[:], in0=xb[:], scalar1=0.0, scalar2=None,
                        op0=OP.min)
                    ex2 = sb.tile([P, H * C1], FP, tag="ex2", bufs=2)
                    nc.scalar.activation(out=ex2[:], in_=mn[:], func=AF.Exp)
                    z1 = sb.tile([P, H * C1], FP, tag="z1", bufs=2)
                    nc.vector.tensor_scalar(
                        out=z1[:], in0=xb[:], scalar1=0.0, scalar2=-1.0,
                        op0=OP.max, op1=OP.add)
                    nc.vector.tensor_tensor(out=z1[:], in0=z1[:], in1=ex2[:],
                                            op=OP.add)
                    # ---- layer-2 node projections for this window ----
                    z1T = sb.tile([P, 2, P], FP, tag="z1T", bufs=2)
                    for ch in range(2):
                        pt = ps.tile([P, P], FP, tag="tpose", space="PSUM", bufs=2)
                        nc.tensor.transpose(out=pt[:],
                                            in_=z1[:, ch * P:(ch + 1) * P],
                                            identity=ident[:])
                        nc.scalar.copy(out=z1T[:, ch, :], in_=pt[:])
                    h2ps = ps.tile([P, C2], FP, tag="sm1", space="PSUM", bufs=2)
                    a2ps = ps.tile([P, 2], FP, tag="sm2", space="PSUM", bufs=1)
                    for ch in range(2):
                        nc.tensor.matmul(out=h2ps[:], lhsT=z1T[:, ch, :],
                                         rhs=W2sb[:, ch, :],
                                         start=(ch == 0), stop=(ch == 1))
                        nc.tensor.matmul(out=a2ps[:], lhsT=z1T[:, ch, :],
                                         rhs=WA2sb[:, ch, :],
                                         start=(ch == 0), stop=(ch == 1))
                    t2 = sb.tile([P, ROW2], FP, tag="t2", bufs=2)
                    nc.gpsimd.memset(t2[:, C2 + 2:ROW2], 0.0)
                    nc.scalar.copy(out=t2[:, 0:C2], in_=h2ps[:])
                    nc.gpsimd.memset(t2[:, C2:C2 + 1], 1.0)
                    nc.vector.tensor_copy(out=t2[:, C2 + 1:C2 + 2],
                                          in_=a2ps[:, 0:1])
                    ad2t = sb.tile([P, ROW2], FP, tag="ad2t", bufs=2)
                    nc.vector.tensor_copy(
                        out=ad2t[:],
                        in_=a2ps[:, 1:2].to_broadcast([P, ROW2]))
                    r0 = wi * P
                    nc.sync.dma_start(out=tab2_loc[r0:r0 + P, :], in_=t2[:])
                    nc.sync.dma_start(out=adtab2[r0:r0 + P, :], in_=ad2t[:])
                sc0 += TG
                t0 += TG

            nc.gpsimd.collective_compute(
                "AllGather", OP.bypass,
                replica_groups=[list(range(NC))],
                ins=[tab2_loc[:]], outs=[tab2_full[:]],
            )

            # ================= Phase D: edge phase, layer 2 =================
            sc0 = 0
            t0 = 0
            for gi, segs in enumerate(geom):
                TG = sum(nt for (_, _, nt) in segs)
                wins = sorted({wi for (wi, _, _) in segs})
                sidx = sb.tile([P, TG * 8], I16, tag="sidx", bufs=2)
                nc.sync.dma_start(out=sidx[:],
                                  in_=srcw[:, sc0 * 8:(sc0 + TG) * 8])
                didx = sb.tile([P, TG * 8], I16, tag="didx", bufs=2)
                nc.sync.dma_start(out=didx[:],
                                  in_=dstw[:, sc0 * 8:(sc0 + TG) * 8])
                dl = sb.tile([P, TG], FP, tag="dl", bufs=2)
                nc.sync.dma_start(out=dl[:], in_=dstloc[:, t0:t0 + TG])

                g2 = sb.tile([P, TG * ROW2], FP, tag="gbuf", bufs=2)
                g2v = g2[:].rearrange("p (t e) -> p t e", e=ROW2)
                off = 0
                for (wi, b, nt) in segs:
                    nc.gpsimd.dma_gather(
                        out_ap=g2v[:, off:off + nt, :],
                        in_ap=tab2_full[b * cfg.BANKP:(b + 1) * cfg.BANKP, :],
                        idxs_ap=sidx[:, off * 8:(off + nt) * 8],
                        num_idxs=nt * P, num_idxs_reg=nt * P,
                        elem_size=ROW2, single_packet=False)
                    off += nt
                d2 = sb.tile([P, TG * ROW2], FP, tag="dbuf", bufs=2)
                d2v = d2[:].rearrange("p (t e) -> p t e", e=ROW2)
                nc.gpsimd.dma_gather(
                    out_ap=d2v, in_ap=adtab2[:], idxs_ap=didx[:],
                    num_idxs=TG * P, num_idxs_reg=TG * P, elem_size=ROW2, single_packet=False)

                e_t = sb.tile([P, TG], FP, tag="e_t2", bufs=2)
                nc.vector.tensor_tensor(
                    out=e_t[:, :, None], in0=g2v[:, :, C2 + 1:C2 + 2],
                    in1=d2v[:, :, 0:1], op=OP.add)
                ee = sb.tile([P, TG], FP, tag="ee2", bufs=2)
                nc.vector.scalar_tensor_tensor(
                    out=ee[:], in0=e_t[:], scalar=0.2, in1=e_t[:],
                    op0=OP.mult, op1=OP.max)
                eex = sb.tile([P, TG], FP, tag="eex2", bufs=2)
                nc.scalar.activation(out=eex[:], in_=ee[:], func=AF.Exp)

                seg_starts = np.cumsum([0] + [nt for (_, _, nt) in segs])
                for wi in wins:
                    tiles = []
                    for si, (w2, b, nt) in enumerate(segs):
                        if w2 == wi:
                            tiles.extend(range(seg_starts[si],
                                               seg_starts[si] + nt))
                    ops_ = ps.tile([P, C2 + 1], FP, tag="sm1", space="PSUM", bufs=2)
                    for j, t in enumerate(tiles):
                        oh = sb.tile([P, P], FP, tag="oh", bufs=4)
                        nc.vector.tensor_scalar(
                            out=oh[:], in0=iota_f[:], scalar1=dl[:, t:t + 1],
                            scalar2=eex[:, t:t + 1], op0=OP.is_equal,
                            op1=OP.mult)
                        nc.tensor.matmul(
                            out=ops_[:], lhsT=oh[:],
                            rhs=g2v[:, t, 0:C2 + 1],
                            start=(j == 0), stop=(j == len(tiles) - 1))
                    den = sb.tile([P, 1], FP, tag="den2")
                    nc.vector.tensor_scalar(
                        out=den[:], in0=ops_[:, C2:C2 + 1], scalar1=1e-16,
                        scalar2=None, op0=OP.add)
                    rden = sb.tile([P, 1], FP, tag="rden2")
                    nc.vector.reciprocal(out=rden[:], in_=den[:])
                    y = sb.tile([P, C2], FP, tag="y")
                    nc.vector.scalar_tensor_tensor(
                        out=y[:], in0=ops_[:, 0:C2], scalar=rden[:, 0:1],
                        in1=b2sb[:], op0=OP.mult, op1=OP.add)
                    mx = sb.tile([P, 1], FP, tag="mx")
                    nc.vector.tensor_reduce(out=mx[:], in_=y[:], op=OP.max,
                                            axis=mybir.AxisListType.X)
                    ys = sb.tile([P, C2], FP, tag="ys")
                    nc.vector.tensor_scalar(
                        out=ys[:], in0=y[:], scalar1=mx[:, 0:1], scalar2=None,
                        op0=OP.subtract)
                    exy = sb.tile([P, C2], FP, tag="exy")
                    sxp = sb.tile([P, 1], FP, tag="sxp")
                    nc.scalar.activation(out=exy[:], in_=ys[:], func=AF.Exp,
                                         accum_out=sxp[:])
                    lse = sb.tile([P, 1], FP, tag="lse")
                    nc.scalar.activation(out=lse[:], in_=sxp[:], func=AF.Ln)
                    o = sb.tile([P, C2], FP, tag="o")
                    nc.vector.tensor_scalar(
                        out=o[:], in0=ys[:], scalar1=lse[:, 0:1], scalar2=None,
                        op0=OP.subtract)
                    r0 = wi * P
                    nc.sync.dma_start(out=out_d[r0:r0 + P, :], in_=o[:])
                sc0 += TG
                t0 += TG

    nc.compile()
    return nc


def host_inputs(cfg, inputs, per_core):
    """Build per-core in_maps from the full problem inputs."""
    x = np.asarray(inputs["x"], np.float32)
    W1 = np.asarray(inputs["W1"], np.float32)
    a_s1 = np.asarray(inputs["att_src1"], np.float32)
    a_d1 = np.asarray(inputs["att_dst1"], np.float32)
    b1 = np.asarray(inputs["b1"], np.float32)
    W2 = np.asarray(inputs["W2"], np.float32)
    a_s2 = np.asarray(inputs["att_src2"], np.float32)
    a_d2 = np.asarray(inputs["att_dst2"], np.float32)
    b2 = np.asarray(inputs["b2"], np.float32)
    H, C1 = cfg.H, cfg.C1

    # fused attention projections: as1 = h1 @ blockdiag(a_src1)
    Ablk = np.zeros((H * C1, 2 * H), np.float32)
    for h in range(H):
        Ablk[h * C1:(h + 1) * C1, h] = a_s1[h]
        Ablk[h * C1:(h + 1) * C1, H + h] = a_d1[h]
    WA1 = (W1 @ Ablk).astype(np.float32)
    WA2 = np.stack([W2 @ a_s2[0], W2 @ a_d2[0]], axis=1).astype(np.float32)
    b1rep = np.tile(b1[None, :], (P, 1)).astype(np.float32)
    b2rep = np.tile(b2[None, :], (P, 1)).astype(np.float32)

    in_maps = []
    for c in range(cfg.NC):
        xs = np.zeros((cfg.SLABP, cfg.F), np.float32)
        xs[0:cfg.SLAB] = x[c * cfg.SLAB:(c + 1) * cfg.SLAB]
        in_maps.append({
            "x_slab": xs, "W1": W1, "WA1": WA1, "b1rep": b1rep,
            "W2": W2, "WA2": WA2, "b2rep": b2rep,
            "srcw": per_core[c]["srcw"], "dstw": per_core[c]["dstw"],
            "dstloc": per_core[c]["dstloc"],
        })
    return in_maps


_CACHE = {}


def prepare(inputs, cfg=None):
    """Build (and cache) the compiled program + per-core inputs."""
    if cfg is None:
        cfg = GATConfig(n_nodes=inputs["x"].shape[0],
                        n_edges=inputs["edge_index"].shape[1])
    key = (cfg.N, cfg.E, cfg.NC, cfg.GRP,
           hash(np.asarray(inputs["edge_index"]).tobytes()))
    if key not in _CACHE:
        geom, per_core = preprocess(cfg, inputs["edge_index"])
        nc = build_program(cfg, geom)
        _CACHE[key] = (cfg, nc, per_core)
    cfg, nc, per_core = _CACHE[key]
    in_maps = host_inputs(cfg, inputs, per_core)
    return cfg, nc, in_maps


def kernel(**inputs):
    cfg, nc, in_maps = prepare(inputs)
    res = run_bass_kernel_spmd(nc, in_maps, core_ids=list(range(cfg.NC)))
    out = np.concatenate(
        [res.results[c]["out"][0:cfg.SLAB] for c in range(cfg.NC)], axis=0)
    return out.astype(np.float32)


def make_runner(cfg, nc, in_maps):
    """Build a persistent jitted callable with device-resident inputs for
    repeat timing.  Returns run() -> list of per-core output arrays."""
    import jax
    from jax.sharding import Mesh, PartitionSpec
    from jax.experimental.shard_map import shard_map
    from concourse import bass2jax, mybir as mb

    bass2jax.install_neuronx_cc_hook()
    n_cores = cfg.NC
    partition_name = (nc.partition_id_tensor.name
                      if nc.partition_id_tensor else None)
    in_names, out_names, out_avals, zero_outs = [], [], [], []
    for alloc in nc.m.functions[0].allocations:
        if not isinstance(alloc, mb.MemoryLocationSet):
            continue
        name = alloc.memorylocations[0].name
        if alloc.kind == "ExternalInput":
            if name != partition_name:
                in_names.append(name)
        elif alloc.kind == "ExternalOutput":
            shape = tuple(alloc.tensor_shape)
            dtype = mb.dt.np(alloc.dtype)
            out_names.append(name)
            out_avals.append(jax.core.ShapedArray(shape, dtype))
            zero_outs.append(np.zeros(shape, dtype))
    n_params = len(in_names)
    all_in = list(in_names) + list(out_names)
    if partition_name is not None:
        all_in.append(partition_name)

    def _body(*args):
        operands = list(args)
        if partition_name is not None:
            operands.append(bass2jax.partition_id_tensor())
        outs = bass2jax._bass_exec_p.bind(
            *operands, out_avals=tuple(out_avals), in_names=tuple(all_in),
            out_names=tuple(out_names), lowering_input_output_aliases=(),
            sim_require_finite=True, sim_require_nnan=True, nc=nc)
        return tuple(outs)

    devices = jax.devices()[:n_cores]
    mesh = Mesh(np.asarray(devices), ("core",))
    in_specs = (PartitionSpec("core"),) * (n_params + len(out_names))
    out_specs = (PartitionSpec("core"),) * len(out_names)
    sharded = jax.jit(shard_map(_body, mesh=mesh, in_specs=in_specs,
                                out_specs=out_specs, check_rep=False),
                      keep_unused=True)
    concat_in = [np.concatenate([np.asarray(in_maps[c][nm])
                                 for c in range(n_cores)], axis=0)
                 for nm in in_names]
    dev_in = [jax.device_put(a) for a in concat_in]
    concat_zeros = [
        jax.device_put(np.zeros((n_cores * z.shape[0], *z.shape[1:]), z.dtype))
        for z in zero_outs]

    def run():
        outs = sharded(*dev_in, *concat_zeros)
        jax.block_until_ready(outs)
        return outs

    return run, out_names, out_avals



# revision 2
# speedup vs baseline: 5.4706x; 5.4706x over previous
"""Trainium2 Bass kernel for a 2-layer GAT (nn_GAT_Net): 50k nodes, 800k
edges, 8 NeuronCores (SPMD, dst-partitioned edge sharding).

Design (per core, slab = 6250 dst nodes):
  - Phase A (node, layer 1): u = x @ W1R for the own slab, where W1R folds a
    per-head invertible rotation M_h with row0 == att_src1[h] into W1.  So
    tab1 rows are exactly 256 bf16 (512B gather descriptors) and the
    per-edge src attention term is u[..., 0] for free.  Per-window dst
    attention values (x @ W1 @ a_dst) stay in SBUF (adwin1).  AllGather
    tab1 (bf16, 25.7MB total).
  - Phase B (edge, layer 1 + node, layer 2): per dst window, batched
    dma_gather of u rows (int16 indices, 2 table banks, round-robin over 4
    SWDGE queues -- queue parallelism is the single biggest hardware lever:
    descriptors of one queue serialize).  Per-edge dst values adE come from
    per-tile PE matmuls  OneHotT @ adwin  accumulated into one PSUM bank and
    read directly by the DVE (no copies); OneHotT is built on the DVE from
    an int8 transposed dst-local map (dlt).  Scores e =
    exp(lrelu(u0 + adE)); messages [u*eex | eex] aggregated per window by
    OneHot^T @ msg matmuls (denominator = the 8 appended eex columns).
    Window tail: normalize, un-rotate via 2x [128x128] bf16 matmuls
    (blockdiag Minv^T), +b1, ELU, then layer-2 projections -> tab2 rows
    (128 bf16 = 256B: [h2 | 1 | as2 | pad]).  AllGather tab2.
  - Phase D (edge, layer 2, H=1): same structure in bf16; eex folded into
    the message rhs; tail: normalize, +b2, log_softmax.
  - Softmax max-subtraction is skipped (scores bounded, exact here).

Self-contained: only numpy + the concourse/bass stack at /opt/trn_rl_repo
(environment infrastructure).  `prepare(inputs, reps=K)` builds a program
that replays the kernel body K times (used by test.py for slope timing).
"""
import sys
import os

for _p in ("/opt/trn_rl_repo",):
    if _p not in sys.path:
        sys.path.insert(0, _p)

import numpy as np
import ml_dtypes

from concourse import bass, mybir, bacc
import concourse.tile as tile
from concourse.masks import make_identity
from concourse.bass_utils import run_bass_kernel_spmd

P = 128
FP = mybir.dt.float32
I16 = mybir.dt.int16
I32 = mybir.dt.int32
BF = mybir.dt.bfloat16
AF = mybir.ActivationFunctionType
OP = mybir.AluOpType
BF_NP = ml_dtypes.bfloat16


class GATConfig:
    def __init__(self, n_nodes=50000, n_edges=800000, n_cores=8, grp=None):
        if grp is None:
            grp = 2
        self.N = n_nodes
        self.E = n_edges
        self.NC = n_cores
        self.F = 256
        self.H = 8
        self.C1 = 32
        self.C2 = 32
        self.SLAB = self.N // self.NC            # real nodes per core
        self.WPC = (self.SLAB + P - 1) // P      # windows per core
        self.SLABP = self.WPC * P                # padded slab
        self.NB = 2                              # src-table banks
        self.BANKP = (self.NC // self.NB) * self.SLABP  # padded rows per bank
        self.ROW1 = 256                          # tab1 row (bf16): u, 512B
        self.ROW2 = 128                          # tab2 row (bf16): 256B
        self.GRP = grp                           # windows per gather group
        self.groups = [list(range(g, min(g + grp, self.WPC)))
                       for g in range(0, self.WPC, grp)]


def _wrap16(idx):
    L = len(idx)
    w = np.asarray(idx, np.int16).reshape(L // 16, 16).T  # [16, L//16]
    return np.tile(w, (8, 1))


def preprocess(cfg, edge_index):
    """Partition edges per core; equalize tile counts across cores.

    per_core[c]: srcw [P, TT*8] i16, dstloc [P, TT] f32, dlt [P, TT*P] bf16.
    """
    SORT_SRC = False
    src = np.asarray(edge_index[0], np.int64)
    dst = np.asarray(edge_index[1], np.int64)
    NC, SLAB, SLABP = cfg.NC, cfg.SLAB, cfg.SLABP
    BANKP = cfg.BANKP
    srcrow = (src // SLAB) * SLABP + (src % SLAB)  # padded-global row
    bank = (srcrow >= BANKP).astype(np.int64)

    edges = [[[None, None] for _ in range(cfg.WPC)] for _ in range(NC)]
    core = dst // SLAB
    for c in range(NC):
        sel = np.nonzero(core == c)[0]
        d_loc = dst[sel] - c * SLAB
        w = d_loc // P
        for wi in range(cfg.WPC):
            wsel = sel[w == wi]
            b = bank[wsel]
            e0 = wsel[b == 0]
            e1 = wsel[b == 1]
            if SORT_SRC:
                e0 = e0[np.argsort(srcrow[e0], kind="stable")]
                e1 = e1[np.argsort(srcrow[e1], kind="stable")]
            edges[c][wi][0] = e0
            edges[c][wi][1] = e1

    geom = []           # per group: list of (window, bank, nt)
    for g in cfg.groups:
        segs = []
        for wi in g:
            for b in range(cfg.NB):
                cnt = max(len(edges[c][wi][b]) for c in range(NC))
                nt = (cnt + P - 1) // P
                if nt > 0:
                    segs.append((wi, b, nt))
        geom.append(segs)

    per_core = []
    for c in range(NC):
        src_cols, dl_cols, dlt_cols = [], [], []
        for gi, g in enumerate(cfg.groups):
            for (wi, b, nt) in geom[gi]:
                e = edges[c][wi][b]
                npad = nt * P - len(e)
                sr = srcrow[e] - b * BANKP
                sr = np.concatenate([sr, np.zeros(npad, np.int64)])
                src_cols.append(_wrap16(sr))
                dlv = (dst[e] - c * SLAB) % P
                dlv = np.concatenate(
                    [dlv.astype(np.float64), np.full(npad, 999.0)])
                dl_cols.append(dlv.reshape(nt, P).T)   # [128, nt]
                # transposed map: column e holds dstloc(e), replicated down
                # all partitions (tile-major layout [P, nt*P]); pad = -1
                dlt8 = np.where(dlv < P, dlv, -1.0).astype(np.int8)
                dlt_cols.append(np.tile(dlt8[None, :], (P, 1)))
        per_core.append({
            "srcw": np.concatenate(src_cols, axis=1).astype(np.int16),
            "dstloc": np.concatenate(dl_cols, axis=1).astype(np.float32),
            "dlt": np.concatenate(dlt_cols, axis=1).astype(np.int8),
        })
    return geom, per_core


def build_program(cfg, geom, reps=1):
    ABL = set()
    NQ = 4
    QSEL = list(range(NQ))
    SP = False
    POHT = False
    OHTR = False
    NC, H, C1, C2, F = cfg.NC, cfg.H, cfg.C1, cfg.C2, cfg.F
    ROW1, ROW2, SLABP, WPC = cfg.ROW1, cfg.ROW2, cfg.SLABP, cfg.WPC
    NPT = NC * SLABP
    TT = sum(nt for segs in geom for (_, _, nt) in segs)
    TGMAX = max(sum(nt for (_, _, nt) in segs) for segs in geom)
    ADGC = TGMAX * H

    nc = bacc.Bacc(None, target_bir_lowering=False, num_devices=NC,
                   num_swdge_queues=NQ)

    x_slab = nc.dram_tensor("x_slab", [SLABP, F], FP, kind="ExternalInput")
    W1R = nc.dram_tensor("W1R", [F, H * C1], FP, kind="ExternalInput")
    WA1 = nc.dram_tensor("WA1", [F, H], FP, kind="ExternalInput")
    b1rep = nc.dram_tensor("b1rep", [P, H * C1], FP, kind="ExternalInput")
    W2 = nc.dram_tensor("W2", [H * C1, C2], FP, kind="ExternalInput")
    WA2 = nc.dram_tensor("WA2", [H * C1, 2], FP, kind="ExternalInput")
    b2rep = nc.dram_tensor("b2rep", [P, C2], FP, kind="ExternalInput")
    Rblk = nc.dram_tensor("Rblk", [2 * P, P], FP, kind="ExternalInput")
    srcw = nc.dram_tensor("srcw", [P, TT * 8], I16, kind="ExternalInput")
    dstloc = nc.dram_tensor("dstloc", [P, TT], FP, kind="ExternalInput")
    dlt = nc.dram_tensor("dlt", [P, TT * P], mybir.dt.int8,
                         kind="ExternalInput")
    out_d = nc.dram_tensor("out", [SLABP, C2], FP, kind="ExternalOutput")

    with tile.TileContext(nc) as tc:
        with (
            tc.tile_pool(name="sbuf", bufs=1) as sb,
            tc.tile_pool(name="psum", bufs=1, space="PSUM") as ps,
            tc.tile_pool(name="dram", bufs=1, space="DRAM") as dp,
        ):
            # ---- constants ----
            ident = sb.tile([P, P], FP, tag="ident")
            make_identity(nc, ident[:])
            ident_b = sb.tile([P, P], BF, tag="ident_b")
            nc.vector.tensor_copy(out=ident_b[:], in_=ident[:])
            iota_i = sb.tile([P, P], I32, tag="iota_i")
            nc.gpsimd.iota(iota_i[:], pattern=[[1, P]], base=0,
                           channel_multiplier=0)
            iota_b = sb.tile([P, P], BF, tag="iota_b")
            nc.vector.tensor_copy(iota_b[:], iota_i[:])
            iotaP_i = sb.tile([P, 1], I32, tag="iotaP_i")
            nc.gpsimd.iota(iotaP_i[:], pattern=[[1, 1]], base=0,
                           channel_multiplier=1)
            iotaP_8 = sb.tile([P, 1], mybir.dt.int8, tag="iotaP_8")
            nc.vector.tensor_copy(iotaP_8[:], iotaP_i[:])
            W1sb = sb.tile([P, 2, F], FP, tag="W1sb")
            nc.sync.dma_start(out=W1sb[:, 0, :], in_=W1R[0:P, :])
            nc.sync.dma_start(out=W1sb[:, 1, :], in_=W1R[P:2 * P, :])
            WA1sb = sb.tile([P, 2, H], FP, tag="WA1sb")
            nc.sync.dma_start(out=WA1sb[:, 0, :], in_=WA1[0:P, :])
            nc.sync.dma_start(out=WA1sb[:, 1, :], in_=WA1[P:2 * P, :])
            W2sb = sb.tile([P, 2, C2], FP, tag="W2sb")
            nc.sync.dma_start(out=W2sb[:, 0, :], in_=W2[0:P, :])
            nc.sync.dma_start(out=W2sb[:, 1, :], in_=W2[P:2 * P, :])
            WA2sb = sb.tile([P, 2, 2], FP, tag="WA2sb")
            nc.sync.dma_start(out=WA2sb[:, 0, :], in_=WA2[0:P, :])
            nc.sync.dma_start(out=WA2sb[:, 1, :], in_=WA2[P:2 * P, :])
            b1sb = sb.tile([P, H * C1], FP, tag="b1sb")
            nc.sync.dma_start(out=b1sb[:], in_=b1rep[:])
            b2sb = sb.tile([P, C2], FP, tag="b2sb")
            nc.sync.dma_start(out=b2sb[:], in_=b2rep[:])
            Rf = sb.tile([P, 2, P], FP, tag="Rf")
            nc.sync.dma_start(out=Rf[:, 0, :], in_=Rblk[0:P, :])
            nc.sync.dma_start(out=Rf[:, 1, :], in_=Rblk[P:2 * P, :])
            Rb = sb.tile([P, 2, P], BF, tag="Rb")
            nc.vector.tensor_copy(out=Rb[:], in_=Rf[:])

            for _rep in range(reps):
                tab1_loc = dp.tile([SLABP, ROW1], BF, tag=f"t1l{_rep}")
                tab1_full = dp.tile([NPT, ROW1], BF, addr_space="Shared",
                                    tag=f"t1f{_rep}")
                tab2_loc = dp.tile([SLABP, ROW2], BF, tag=f"t2l{_rep}")
                tab2_full = dp.tile([NPT, ROW2], BF, addr_space="Shared",
                                    tag=f"t2f{_rep}")
                # per-window dst attention values, SBUF-resident
                adwin1 = sb.tile([P, WPC, H], BF, tag="adwin1")
                adwin2 = sb.tile([P, WPC, 1], BF, tag="adwin2")

                # ============ Phase A: node phase, layer 1 ============
                for i in range(WPC):
                    r0 = i * P
                    xx = sb.tile([P, F], FP, tag="xx", bufs=2)
                    nc.sync.dma_start(out=xx[:], in_=x_slab[r0:r0 + P, :])
                    xT = sb.tile([P, 2, P], FP, tag="xT", bufs=2)
                    for ch in range(2):
                        pt = ps.tile([P, P], FP, tag="tpose", space="PSUM",
                                     bufs=1)
                        nc.tensor.transpose(
                            out=pt[:], in_=xx[:, ch * P:(ch + 1) * P],
                            identity=ident[:])
                        nc.scalar.copy(out=xT[:, ch, :], in_=pt[:])
                    hpsb = ps.tile([P, ROW1 + H], FP, tag="big",
                                   space="PSUM", bufs=2)
                    hps = hpsb[:, 0:H * C1]
                    apsb = ps.tile([P, 33], FP, tag="small", space="PSUM",
                                   bufs=2)
                    aps = apsb[:, 0:H]
                    for ch in range(2):
                        nc.tensor.matmul(out=hps, lhsT=xT[:, ch, :],
                                         rhs=W1sb[:, ch, :],
                                         start=(ch == 0), stop=(ch == 1))
                        nc.tensor.matmul(out=aps, lhsT=xT[:, ch, :],
                                         rhs=WA1sb[:, ch, :],
                                         start=(ch == 0), stop=(ch == 1))
                    t1 = sb.tile([P, ROW1], BF, tag="t1", bufs=2)
                    nc.scalar.copy(out=t1[:], in_=hps)
                    nc.scalar.copy(out=adwin1[:, i, :], in_=aps)
                    nc.sync.dma_start(out=tab1_loc[r0:r0 + P, :], in_=t1[:])

                if "coll" in ABL:
                    nc.sync.dma_start(out=tab1_full[0:SLABP, :],
                                      in_=tab1_loc[:])
                else:
                    nc.gpsimd.collective_compute(
                        "AllGather", OP.bypass,
                        replica_groups=[list(range(NC))],
                        ins=[tab1_loc[:]], outs=[tab1_full[:]],
                    )

                # ===== Phase B: edge phase layer 1 + node phase layer 2 ====
                sidx_all = sb.tile([P, TT * 8], I16, tag="sidx_all")
                nc.sync.dma_start(out=sidx_all[:], in_=srcw[:])
                dl_all = sb.tile([P, TT], FP, tag="dl_all")
                nc.sync.dma_start(out=dl_all[:], in_=dstloc[:])
                qrr = 0
                sc0 = 0
                t0 = 0
                for gi, segs in enumerate(geom):
                    TG = sum(nt for (_, _, nt) in segs)
                    wins = sorted({wi for (wi, _, _) in segs})
                    sidx = sidx_all[:, t0 * 8:(t0 + TG) * 8]
                    dl = dl_all[:, t0:t0 + TG]
                    dltg = sb.tile([P, TG * P], mybir.dt.int8,
                                   tag="dltg", bufs=2)
                    nc.sync.dma_start(out=dltg[:],
                                      in_=dlt[:, t0 * P:(t0 + TG) * P])
                    dltv = dltg[:].rearrange("p (t e) -> p t e", e=P)

                    gbuf = sb.tile([P, TG * ROW1], BF, tag="gbuf", bufs=2)
                    gv = gbuf[:].rearrange("p (t e) -> p t e", e=ROW1)
                    off = 0
                    for si, (wi, b, nt) in enumerate(segs):
                        if "gather" in ABL:
                            nc.sync.dma_start(
                                out=gv[:, off:off + nt, :],
                                in_=tab1_full[0:P * nt, :].rearrange(
                                    "(t p) e -> p t e", p=P))
                        else:
                            nc.gpsimd.dma_gather(
                                out_ap=gv[:, off:off + nt, :],
                                in_ap=tab1_full[b * cfg.BANKP:
                                                (b + 1) * cfg.BANKP, :],
                                idxs_ap=sidx[:, off * 8:(off + nt) * 8],
                                num_idxs=nt * P, num_idxs_reg=nt * P,
                                elem_size=ROW1, single_packet=SP,
                                queue_num=QSEL[qrr % len(QSEL)])
                        qrr += 1
                        off += nt

                    # transposed one-hots for the whole group
                    ohT = sb.tile([P, TG * P], BF, tag="ohT", bufs=2)
                    _eng = nc.gpsimd if POHT else nc.vector
                    _eng.tensor_tensor(
                        out=ohT[:].rearrange("p (t e) -> p t e", e=P),
                        in0=iotaP_8[:, None, 0:1].to_broadcast([P, TG, P]),
                        in1=dltv, op=OP.is_equal)
                    ohTv = ohT[:].rearrange("p (t e) -> p t e", e=P)

                    # per-edge ad via one-hot^T @ adwin, all tiles into one
                    # PSUM bank; the score add below reads PSUM directly
                    seg_starts = np.cumsum([0] + [nt for (_, _, nt) in segs])
                    adg = ps.tile([P, ADGC], FP, tag="adg", space="PSUM",
                                  bufs=1)
                    for si, (wi, b, nt) in enumerate(segs):
                        for t in range(seg_starts[si], seg_starts[si] + nt):
                            nc.tensor.matmul(out=adg[:, t * H:(t + 1) * H],
                                             lhsT=ohTv[:, t, :],
                                             rhs=adwin1[:, wi, :],
                                             start=True, stop=True)

                    # scores for the whole group: e = u0(src) + adE
                    e_t = sb.tile([P, TG * H], FP, tag="e_t", bufs=2)
                    nc.vector.tensor_tensor(
                        out=e_t[:].rearrange("p (t h) -> p t h", h=H)[
                            :, :, :, None],
                        in0=gv[:].rearrange("p t (h c) -> p t h c", c=C1)[
                            :, :, :, 0:1],
                        in1=adg[:, 0:TG * H].rearrange(
                            "p (t h) -> p t h", h=H)[:, :, :, None],
                        op=OP.add)
                    ee = sb.tile([P, TG * H], FP, tag="ee", bufs=2)
                    nc.vector.scalar_tensor_tensor(
                        out=ee[:], in0=e_t[:], scalar=0.2, in1=e_t[:],
                        op0=OP.mult, op1=OP.max)
                    eex = sb.tile([P, TG * H], FP, tag="eex", bufs=2)
                    nc.scalar.activation(out=eex[:], in_=ee[:], func=AF.Exp)
                    eexv = eex[:].rearrange("p (t h) -> p t h", h=H)

                    # messages for the whole group: [u * eex | eex]
                    msg = sb.tile([P, TG * (ROW1 + H)], BF, tag="msg", bufs=2)
                    msgv = msg[:].rearrange("p (t e) -> p t e", e=ROW1 + H)
                    nc.vector.tensor_tensor(
                        out=msgv[:, :, 0:ROW1].rearrange(
                            "p t (h c) -> p t h c", c=C1),
                        in0=gv[:].rearrange("p t (h c) -> p t h c", c=C1),
                        in1=eexv[:, :, :, None].to_broadcast([P, TG, H, C1]),
                        op=OP.mult)
                    nc.scalar.copy(out=msgv[:, :, ROW1:ROW1 + H],
                                   in_=eexv)

                    ohg = sb.tile([P, TG * P], BF, tag="ohg", bufs=2)
                    ohg_v = ohg[:].rearrange("p (t e) -> p t e", e=P)
                    if OHTR:
                        for _t in range(TG):
                            ptb = ps.tile([P, P], BF, tag="tposeb",
                                          space="PSUM", bufs=2)
                            nc.tensor.transpose(out=ptb[:],
                                                in_=ohTv[:, _t, :],
                                                identity=ident_b[:])
                            nc.scalar.copy(out=ohg_v[:, _t, :], in_=ptb[:])
                    else:
                        nc.vector.tensor_tensor(
                            out=ohg_v,
                            in0=iota_b[:, None, 0:P].to_broadcast([P, TG, P]),
                            in1=dl[:, :, None].to_broadcast([P, TG, P]),
                            op=OP.is_equal)

                    # per-window aggregation + tail
                    for wi in wins:
                        tiles = []
                        for si, (w2, b, nt) in enumerate(segs):
                            if w2 == wi:
                                tiles.extend(range(seg_starts[si],
                                                   seg_starts[si] + nt))
                        aggps = ps.tile([P, ROW1 + H], FP, tag="big",
                                        space="PSUM", bufs=2)
                        for j, t in enumerate(tiles):
                            nc.tensor.matmul(
                                out=aggps[:], lhsT=ohg_v[:, t, :],
                                rhs=msgv[:, t, :],
                                start=(j == 0), stop=(j == len(tiles) - 1))
                        # ---- window tail ----
                        aggu = aggps[:, 0:ROW1].rearrange(
                            "p (h c) -> p h c", c=C1)
                        den = sb.tile([P, H], FP, tag="den", bufs=2)
                        nc.vector.tensor_scalar(
                            out=den[:], in0=aggps[:, ROW1:ROW1 + H],
                            scalar1=1e-16, scalar2=None, op0=OP.add)
                        rden = sb.tile([P, H], FP, tag="rden", bufs=2)
                        nc.vector.reciprocal(out=rden[:], in_=den[:])
                        normu = sb.tile([P, ROW1], BF, tag="normu", bufs=2)
                        nc.vector.tensor_tensor(
                            out=normu[:].rearrange("p (h c) -> p h c", c=C1),
                            in0=aggu,
                            in1=rden[:, :, None].to_broadcast([P, H, C1]),
                            op=OP.mult)
                        # un-rotate: h = (normu @ blockdiag(Minv^T))
                        unT = sb.tile([P, 2, P], BF, tag="unT", bufs=2)
                        for ch in range(2):
                            pt = ps.tile([P, P], BF, tag="tposeb",
                                         space="PSUM", bufs=2)
                            nc.tensor.transpose(
                                out=pt[:], in_=normu[:, ch * P:(ch + 1) * P],
                                identity=ident_b[:])
                            nc.scalar.copy(out=unT[:, ch, :], in_=pt[:])
                        hps2b = ps.tile([P, ROW1 + H], FP, tag="big",
                                        space="PSUM", bufs=2)
                        hps2 = hps2b[:, 0:H * C1]
                        for ch in range(2):
                            nc.tensor.matmul(
                                out=hps2[:, ch * P:(ch + 1) * P],
                                lhsT=unT[:, ch, :], rhs=Rb[:, ch, :],
                                start=True, stop=True)
                        xb = sb.tile([P, H * C1], FP, tag="xb", bufs=2)
                        nc.vector.tensor_tensor(out=xb[:], in0=hps2,
                                                in1=b1sb[:], op=OP.add)
                        mn = sb.tile([P, H * C1], FP, tag="mn", bufs=2)
                        nc.vector.tensor_scalar(
                            out=mn[:], in0=xb[:], scalar1=0.0, scalar2=None,
                            op0=OP.min)
                        ex2 = sb.tile([P, H * C1], FP, tag="ex2", bufs=2)
                        nc.scalar.activation(out=ex2[:], in_=mn[:],
                                             func=AF.Exp)
                        z1 = sb.tile([P, H * C1], FP, tag="z1", bufs=2)
                        nc.vector.tensor_scalar(
                            out=z1[:], in0=xb[:], scalar1=0.0, scalar2=-1.0,
                            op0=OP.max, op1=OP.add)
                        nc.vector.tensor_tensor(out=z1[:], in0=z1[:],
                                                in1=ex2[:], op=OP.add)
                        # ---- layer-2 node projections for this window ----
                        z1T = sb.tile([P, 2, P], FP, tag="z1T", bufs=2)
                        for ch in range(2):
                            pt = ps.tile([P, P], FP, tag="tpose",
                                         space="PSUM", bufs=1)
                            nc.tensor.transpose(
                                out=pt[:], in_=z1[:, ch * P:(ch + 1) * P],
                                identity=ident[:])
                            nc.scalar.copy(out=z1T[:, ch, :], in_=pt[:])
                        h2psb = ps.tile([P, 33], FP, tag="small",
                                        space="PSUM", bufs=2)
                        h2ps = h2psb[:, 0:C2]
                        a2psb = ps.tile([P, 33], FP, tag="small",
                                        space="PSUM", bufs=2)
                        a2ps = a2psb[:, 0:2]
                        for ch in range(2):
                            nc.tensor.matmul(out=h2ps, lhsT=z1T[:, ch, :],
                                             rhs=W2sb[:, ch, :],
                                             start=(ch == 0), stop=(ch == 1))
                            nc.tensor.matmul(out=a2ps, lhsT=z1T[:, ch, :],
                                             rhs=WA2sb[:, ch, :],
                                             start=(ch == 0), stop=(ch == 1))
                        t2 = sb.tile([P, ROW2], BF, tag="t2", bufs=2)
                        nc.gpsimd.memset(t2[:, C2 + 2:ROW2], 0.0)
                        nc.scalar.copy(out=t2[:, 0:C2], in_=h2ps)
                        nc.gpsimd.memset(t2[:, C2:C2 + 1], 1.0)
                        nc.scalar.copy(out=t2[:, C2 + 1:C2 + 2],
                                       in_=a2ps[:, 0:1])
                        nc.scalar.copy(out=adwin2[:, wi, :],
                                       in_=a2ps[:, 1:2])
                        r0 = wi * P
                        nc.sync.dma_start(out=tab2_loc[r0:r0 + P, :],
                                          in_=t2[:])
                    sc0 += TG
                    t0 += TG

                if "coll" in ABL:
                    nc.sync.dma_start(out=tab2_full[0:SLABP, :],
                                      in_=tab2_loc[:])
                else:
                    nc.gpsimd.collective_compute(
                        "AllGather", OP.bypass,
                        replica_groups=[list(range(NC))],
                        ins=[tab2_loc[:]], outs=[tab2_full[:]],
                    )

                # ============ Phase D: edge phase, layer 2 ============
                sc0 = 0
                t0 = 0
                for gi, segs in enumerate(geom):
                    TG = sum(nt for (_, _, nt) in segs)
                    wins = sorted({wi for (wi, _, _) in segs})
                    sidx = sidx_all[:, t0 * 8:(t0 + TG) * 8]
                    dl = dl_all[:, t0:t0 + TG]
                    dltg = sb.tile([P, TG * P], mybir.dt.int8,
                                   tag="dltg", bufs=2)
                    nc.sync.dma_start(out=dltg[:],
                                      in_=dlt[:, t0 * P:(t0 + TG) * P])
                    dltv = dltg[:].rearrange("p (t e) -> p t e", e=P)

                    g2 = sb.tile([P, TG * ROW2], BF, tag="g2buf", bufs=2)
                    g2v = g2[:].rearrange("p (t e) -> p t e", e=ROW2)
                    off = 0
                    for si, (wi, b, nt) in enumerate(segs):
                        if "gather2" in ABL:
                            nc.sync.dma_start(
                                out=g2v[:, off:off + nt, :],
                                in_=tab2_full[0:P * nt, :].rearrange(
                                    "(t p) e -> p t e", p=P))
                        else:
                            nc.gpsimd.dma_gather(
                                out_ap=g2v[:, off:off + nt, :],
                                in_ap=tab2_full[b * cfg.BANKP:
                                                (b + 1) * cfg.BANKP, :],
                                idxs_ap=sidx[:, off * 8:(off + nt) * 8],
                                num_idxs=nt * P, num_idxs_reg=nt * P,
                                elem_size=ROW2, single_packet=SP,
                                queue_num=QSEL[qrr % len(QSEL)])
                        qrr += 1
                        off += nt

                    ohT = sb.tile([P, TG * P], BF, tag="ohT", bufs=2)
                    _eng = nc.gpsimd if POHT else nc.vector
                    _eng.tensor_tensor(
                        out=ohT[:].rearrange("p (t e) -> p t e", e=P),
                        in0=iotaP_8[:, None, 0:1].to_broadcast([P, TG, P]),
                        in1=dltv, op=OP.is_equal)
                    ohTv = ohT[:].rearrange("p (t e) -> p t e", e=P)

                    seg_starts = np.cumsum([0] + [nt for (_, _, nt) in segs])
                    adg = ps.tile([P, ADGC], FP, tag="adg", space="PSUM",
                                  bufs=1)
                    for si, (wi, b, nt) in enumerate(segs):
                        for t in range(seg_starts[si], seg_starts[si] + nt):
                            nc.tensor.matmul(out=adg[:, t:t + 1],
                                             lhsT=ohTv[:, t, :],
                                             rhs=adwin2[:, wi, :],
                                             start=True, stop=True)

                    e_t = sb.tile([P, TG], FP, tag="e_t2", bufs=2)
                    nc.vector.tensor_tensor(
                        out=e_t[:, :, None], in0=g2v[:, :, C2 + 1:C2 + 2],
                        in1=adg[:, 0:TG, None], op=OP.add)
                    ee = sb.tile([P, TG], FP, tag="ee2", bufs=2)
                    nc.vector.scalar_tensor_tensor(
                        out=ee[:], in0=e_t[:], scalar=0.2, in1=e_t[:],
                        op0=OP.mult, op1=OP.max)
                    eex = sb.tile([P, TG], FP, tag="eex2", bufs=2)
                    nc.scalar.activation(out=eex[:], in_=ee[:], func=AF.Exp)

                    oh2 = sb.tile([P, TG * P], BF, tag="ohg", bufs=2)
                    oh2v = oh2[:].rearrange("p (t e) -> p t e", e=P)
                    if OHTR:
                        for _t in range(TG):
                            ptb = ps.tile([P, P], BF, tag="tposeb",
                                          space="PSUM", bufs=2)
                            nc.tensor.transpose(out=ptb[:],
                                                in_=ohTv[:, _t, :],
                                                identity=ident_b[:])
                            nc.scalar.copy(out=oh2v[:, _t, :], in_=ptb[:])
                    else:
                        nc.vector.tensor_tensor(
                            out=oh2v,
                            in0=iota_b[:, None, 0:P].to_broadcast([P, TG, P]),
                            in1=dl[:, :, None].to_broadcast([P, TG, P]),
                            op=OP.is_equal)
                    msg2 = sb.tile([P, TG * (C2 + 1)], BF, tag="msg2",
                                   bufs=2)
                    msg2v = msg2[:].rearrange("p (t e) -> p t e", e=C2 + 1)
                    nc.vector.tensor_tensor(
                        out=msg2v, in0=g2v[:, :, 0:C2 + 1],
                        in1=eex[:, :, None].to_broadcast([P, TG, C2 + 1]),
                        op=OP.mult)

                    for wi in wins:
                        tiles = []
                        for si, (w2, b, nt) in enumerate(segs):
                            if w2 == wi:
                                tiles.extend(range(seg_starts[si],
                                                   seg_starts[si] + nt))
                        ops_ = ps.tile([P, 33], FP, tag="small",
                                       space="PSUM", bufs=2)
                        for j, t in enumerate(tiles):
                            nc.tensor.matmul(
                                out=ops_[:, 0:C2 + 1], lhsT=oh2v[:, t, :],
                                rhs=msg2v[:, t, :],
                                start=(j == 0), stop=(j == len(tiles) - 1))
                        den = sb.tile([P, 1], FP, tag="den2")
                        nc.vector.tensor_scalar(
                            out=den[:], in0=ops_[:, C2:C2 + 1], scalar1=1e-16,
                            scalar2=None, op0=OP.add)
                        rden = sb.tile([P, 1], FP, tag="rden2")
                        nc.vector.reciprocal(out=rden[:], in_=den[:])
                        y = sb.tile([P, C2], FP, tag="y")
                        nc.vector.scalar_tensor_tensor(
                            out=y[:], in0=ops_[:, 0:C2], scalar=rden[:, 0:1],
                            in1=b2sb[:], op0=OP.mult, op1=OP.add)
                        mx = sb.tile([P, 1], FP, tag="mx")
                        nc.vector.tensor_reduce(out=mx[:], in_=y[:],
                                                op=OP.max,
                                                axis=mybir.AxisListType.X)
                        ys = sb.tile([P, C2], FP, tag="ys")
                        nc.vector.tensor_scalar(
                            out=ys[:], in0=y[:], scalar1=mx[:, 0:1],
                            scalar2=None, op0=OP.subtract)
                        exy = sb.tile([P, C2], FP, tag="exy")
                        sxp = sb.tile([P, 1], FP, tag="sxp")
                        nc.scalar.activation(out=exy[:], in_=ys[:],
                                             func=AF.Exp, accum_out=sxp[:])
                        lse = sb.tile([P, 1], FP, tag="lse")
                        nc.scalar.activation(out=lse[:], in_=sxp[:],
                                             func=AF.Ln)
                        o = sb.tile([P, C2], FP, tag="o")
                        nc.vector.tensor_scalar(
                            out=o[:], in0=ys[:], scalar1=lse[:, 0:1],
                            scalar2=None, op0=OP.subtract)
                        r0 = wi * P
                        nc.sync.dma_start(out=out_d[r0:r0 + P, :], in_=o[:])
                    sc0 += TG
                    t0 += TG

    nc.compile()
    return nc


def host_inputs(cfg, inputs, per_core):
    x = np.asarray(inputs["x"], np.float32)
    W1 = np.asarray(inputs["W1"], np.float32)
    a_s1 = np.asarray(inputs["att_src1"], np.float32)
    a_d1 = np.asarray(inputs["att_dst1"], np.float32)
    b1 = np.asarray(inputs["b1"], np.float32)
    W2 = np.asarray(inputs["W2"], np.float32)
    a_s2 = np.asarray(inputs["att_src2"], np.float32)
    a_d2 = np.asarray(inputs["att_dst2"], np.float32)
    b2 = np.asarray(inputs["b2"], np.float32)
    H, C1 = cfg.H, cfg.C1

    # per-head rotation M_h (row 0 == a_src1[h]) folded into W1; inverse
    # transpose blockdiag for the post-aggregation un-rotation.
    BD = np.zeros((H * C1, H * C1), np.float64)
    Rblk = np.zeros((2 * P, P), np.float64)
    for h in range(H):
        a = a_s1[h].astype(np.float64)
        Q, _ = np.linalg.qr(a[:, None], mode="complete")
        M = np.vstack([a[None, :], Q[:, 1:].T])
        Minv = np.linalg.inv(M)
        BD[h * C1:(h + 1) * C1, h * C1:(h + 1) * C1] = M.T
        r0 = h * C1
        Rblk[r0:r0 + C1, (r0 % P):(r0 % P) + C1] = Minv.T
    W1R = (W1.astype(np.float64) @ BD).astype(np.float32)
    WA1 = (W1 @ (np.kron(np.eye(H, dtype=np.float32), np.ones((C1, 1),
                 np.float32)) * np.repeat(a_d1, 1, axis=0).reshape(-1, 1)
                 ).reshape(H * C1, H)
           ).astype(np.float32)
    # ^ WA1[f, h] = sum_c W1[f, h*C1+c] * a_d1[h, c]
    WA2 = np.stack([W2 @ a_s2[0], W2 @ a_d2[0]], axis=1).astype(np.float32)
    b1rep = np.tile(b1[None, :], (P, 1)).astype(np.float32)
    b2rep = np.tile(b2[None, :], (P, 1)).astype(np.float32)

    in_maps = []
    for c in range(cfg.NC):
        xs = np.zeros((cfg.SLABP, cfg.F), np.float32)
        xs[0:cfg.SLAB] = x[c * cfg.SLAB:(c + 1) * cfg.SLAB]
        in_maps.append({
            "x_slab": xs, "W1R": W1R, "WA1": WA1, "b1rep": b1rep,
            "W2": W2, "WA2": WA2, "b2rep": b2rep,
            "Rblk": Rblk.astype(np.float32),
            "srcw": per_core[c]["srcw"], "dstloc": per_core[c]["dstloc"],
            "dlt": per_core[c]["dlt"],
        })
    return in_maps


_CACHE = {}


def prepare(inputs, cfg=None, reps=1):
    if cfg is None:
        cfg = GATConfig(n_nodes=inputs["x"].shape[0],
                        n_edges=inputs["edge_index"].shape[1])
    key = (cfg.N, cfg.E, cfg.NC, cfg.GRP, reps,
           hash(np.asarray(inputs["edge_index"]).tobytes()))
    if key not in _CACHE:
        geom, per_core = preprocess(cfg, inputs["edge_index"])
        nc = build_program(cfg, geom, reps=reps)
        _CACHE[key] = (cfg, nc, per_core)
    cfg, nc, per_core = _CACHE[key]
    in_maps = host_inputs(cfg, inputs, per_core)
    return cfg, nc, in_maps


def kernel(**inputs):
    cfg, nc, in_maps = prepare(inputs)
    res = run_bass_kernel_spmd(nc, in_maps, core_ids=list(range(cfg.NC)))
    out = np.concatenate(
        [res.results[c]["out"][0:cfg.SLAB] for c in range(cfg.NC)], axis=0)
    return out.astype(np.float32)


def make_runner(cfg, nc, in_maps):
    """Persistent jitted callable with device-resident inputs for timing."""
    import jax
    from jax.sharding import Mesh, PartitionSpec
    from jax.experimental.shard_map import shard_map
    from concourse import bass2jax, mybir as mb

    bass2jax.install_neuronx_cc_hook()
    n_cores = cfg.NC
    partition_name = (nc.partition_id_tensor.name
                      if nc.partition_id_tensor else None)
    in_names, out_names, out_avals, zero_outs = [], [], [], []
    for alloc in nc.m.functions[0].allocations:
        if not isinstance(alloc, mb.MemoryLocationSet):
            continue
        name = alloc.memorylocations[0].name
        if alloc.kind == "ExternalInput":
            if name != partition_name:
                in_names.append(name)
        elif alloc.kind == "ExternalOutput":
            shape = tuple(alloc.tensor_shape)
            dtype = mb.dt.np(alloc.dtype)
            out_names.append(name)
            out_avals.append(jax.core.ShapedArray(shape, dtype))
            zero_outs.append(np.zeros(shape, dtype))
    n_params = len(in_names)
    all_in = list(in_names) + list(out_names)
    if partition_name is not None:
        all_in.append(partition_name)

    def _body(*args):
        operands = list(args)
        if partition_name is not None:
            operands.append(bass2jax.partition_id_tensor())
        outs = bass2jax._bass_exec_p.bind(
            *operands, out_avals=tuple(out_avals), in_names=tuple(all_in),
            out_names=tuple(out_names), lowering_input_output_aliases=(),
            sim_require_finite=False, sim_require_nnan=False, nc=nc)
        return tuple(outs)

    devices = jax.devices()[:n_cores]
    mesh = Mesh(np.asarray(devices), ("core",))
    in_specs = (PartitionSpec("core"),) * (n_params + len(out_names))
    out_specs = (PartitionSpec("core"),) * len(out_names)
    sharded = jax.jit(shard_map(_body, mesh=mesh, in_specs=in_specs,
                                out_specs=out_specs, check_rep=False),
                      keep_unused=True)
    concat_in = [np.concatenate([np.asarray(in_maps[c][nm])
                                 for c in range(n_cores)], axis=0)
                 for nm in in_names]
    dev_in = [jax.device_put(a) for a in concat_in]
    concat_zeros = [
        jax.device_put(np.zeros((n_cores * z.shape[0], *z.shape[1:]),
                                z.dtype))
        for z in zero_outs]

    def run():
        outs = sharded(*dev_in, *concat_zeros)
        jax.block_until_ready(outs)
        return outs

    return run, out_names, out_avals

